# revision 9
# baseline (speedup 1.0000x reference)
"""Causal multi-head attention (B=2, H=12, T=2048, D=64) on 8 Trainium2 NeuronCores.

Sharding: the 24 (batch, head) pairs are split 3-per-core across 8 cores.

Per head the device kernel works panel-major in transposed-score layout: the
16 q-tiles form 4 panels of 512 q columns. For panel p, kv strips j=0..4p+3
(width 512 for j<=4p, then 384/256/128) are processed as row-tiled PAIRS:

    S^T strips = K_j @ Q_panel   (PE, K=64 contraction). The two strips of a
        pair run CONCURRENTLY on array rows 0-63 / 64-127 (Q/K host-duplicated
        across both partition halves), writing the two 512-col banks of one
        [128,1024] PSUM mega tile.
    P^T = exp(S^T/8)             usually ONE instruction per pair, greedily
        load-balanced between ACT (true exp) and DVE (1-pass Schraudolph
        fast-exp: i16 = S*(128/ln2)+B, bitcast fp16, ~3% max err — the
        softmax ratio cancels most of it).
    diag blocks masked in-place on GPSIMD (affine_select, keep q >= kv)
    O'^T[65, 512] += [V_j | ones] @ P^T strip   (PE, N<=512 wide; row 64
        accumulates softmax denominators)
    tail per panel: O'^T -> SBUF fp16, 4 PE transposes (fp16, 66-col stride
        for PSUM alignment), DVE reciprocal of the denominator column,
        broadcast multiply into a per-head fp16 output buffer, one
        partition-major DMA out per head (host untangles the layout).

`repeat` > 1 wraps the body in a hardware For_i loop (timing aid only).

Self-contained: only imports numpy + the installed concourse/bass stack.
"""

import math
import os
import numpy as np

B, H, T, D = 2, 12, 2048, 64
NCORES = 8
HPC = (B * H) // NCORES      # heads per core = 3
NQT = T // 128               # 16 q tiles of 128 rows
NPANEL = 4                   # panels of 4 q-tiles (512 q cols)
SCALE = 1.0 / 8.0            # 1/sqrt(D)

# Schraudolph fast-exp constants (fp16 bit pattern via int16):
#   i16 = round(S_raw * (2^10/ln2 * SCALE) + B_EXP);  fp16<bits=i16> ~ exp(S_raw/8)
A_EXP = 1024.0 / math.log(2.0) * SCALE
B_EXP = 15315.5

# estimated per-op engine costs (ns) for the ACT/DVE load balancer
ACT_COL = 1.0 / 1.2
DVE_COL = 1.0 / 0.96
ACT_FIX = 330.0
DVE_FIX = 260.0

_cache = {}


def _width(p, j):
    return 512 if j <= 4 * p else (4 * p + 4 - j) * 128


def build_program(io_bufs=2, mega_bufs=3, oq_bufs=1, ot_bufs=1, tail_delay=1,
                  repeat=1):
    import concourse.bacc as bacc
    import concourse.bass as bass
    import concourse.mybir as mybir
    import concourse.tile as tile
    from concourse.masks import make_identity

    f16 = mybir.dt.float16
    f32 = mybir.dt.float32
    i16 = mybir.dt.int16
    Exp = mybir.ActivationFunctionType.Exp
    Alu = mybir.AluOpType

    nc = bacc.Bacc(None)
    # q/k stored transposed [d, t], duplicated across both partition halves
    q_d = nc.dram_tensor("qd", [HPC, 128, T], f16, kind="ExternalInput")
    k_d = nc.dram_tensor("kd", [HPC, 128, T], f16, kind="ExternalInput")
    v_d = nc.dram_tensor("v", [HPC, 128, NQT, D + 1], f16, kind="ExternalInput")
    # partition-major: [head, q-row-in-tile, q-tile, d]; host untangles
    o_d = nc.dram_tensor("out", [HPC, 128, NQT, D], f16, kind="ExternalOutput")

    strip_off = {}
    pt_cols = {}
    for p in range(NPANEL):
        off = 0
        for j in range(4 * p + 4):
            strip_off[(p, j)] = off
            off += _width(p, j)
        pt_cols[p] = off

    # units: consecutive strip pairs
    unit_list = []
    for h in range(HPC):
        for p in range(NPANEL):
            js = list(range(4 * p + 4))
            for g in range(0, len(js), 2):
                unit_list.append((h, p, tuple(js[g:g + 2])))

    with tile.TileContext(nc) as tc:
        with (
            tc.tile_pool(name="consts", bufs=1) as consts,
            tc.tile_pool(name="qk", bufs=io_bufs) as qk,
            tc.tile_pool(name="vpool", bufs=io_bufs) as vpool,
            tc.tile_pool(name="ptpool", bufs=2) as ptpool,
            tc.tile_pool(name="odp", bufs=2) as odp,
            tc.tile_pool(name="rp", bufs=2) as rp,
            tc.tile_pool(name="finp", bufs=2) as finp,
            tc.tile_pool(name="smf", bufs=mega_bufs, space="PSUM") as smf,
            tc.tile_pool(name="oqp", bufs=oq_bufs, space="PSUM") as oqp,
            tc.tile_pool(name="otp", bufs=ot_bufs, space="PSUM") as otp,
        ):
            ident = consts.tile([128, 128], f16)
            make_identity(nc, ident[:])
            # Warm the ACT exp table set while the first DMAs are in flight.
            warm = consts.tile([128, 1], f32)
            nc.scalar.activation(warm[:], ident[:, 0:1], Exp)

            def emit_body():
                heads = {}
                panels = {}
                bal = {"A": 0.0, "D": 0.0}
                pending = []

                def pick(act_cost, dve_cost):
                    if bal["A"] + act_cost <= bal["D"] + dve_cost:
                        bal["A"] += act_cost
                        return "A"
                    bal["D"] += dve_cost
                    return "D"

                def emit_loads(h, chunks=1):
                    qt = qk.tile([128, T], f16, tag="qt")
                    kt = qk.tile([128, T], f16, tag="kt")
                    cw = T // chunks
                    for c in range(0, T, cw):
                        nc.sync.dma_start(kt[:, c:c + cw], k_d[h, :, c:c + cw])
                        nc.sync.dma_start(qt[:, c:c + cw], q_d[h, :, c:c + cw])
                    vp = vpool.tile([128, NQT, D + 1], f16)
                    nc.sync.dma_start(vp[:], v_d[h])
                    fin = finp.tile([128, NQT, D], f16)
                    heads[h] = {"qt": qt, "kt": kt, "vp": vp, "fin": fin}

                def emit_scores(u):
                    h, p, js = u
                    qbase = 512 * p
                    hd = heads[h]
                    if (h, p) not in panels:
                        panels[(h, p)] = {
                            "pt": ptpool.tile([128, pt_cols[p]], f16,
                                              tag="pt", name="pt"),
                            "oq": oqp.tile([D + 1, 512], f32, tag="oq",
                                           name="oq"),
                            "sm": {},
                        }
                    pan = panels[(h, p)]
                    sm = smf.tile([128, 1024], f32, tag="sm", name="sm")
                    for side, j in enumerate(js):
                        w = _width(p, j)
                        rows = slice(64 * side, 64 * side + 64)
                        c0 = 512 * side
                        nc.tensor.matmul(
                            sm[:, c0:c0 + w],
                            hd["kt"][rows, j * 128:(j + 1) * 128],
                            hd["qt"][rows, qbase + 512 - w:qbase + 512],
                        )
                        pan["sm"][j] = (sm, c0, w)

                def emit_exps(u):
                    h, p, js = u
                    pan = panels[(h, p)]
                    w0 = _width(p, js[0])
                    w1 = _width(p, js[-1]) if len(js) > 1 else 0
                    off = strip_off[(p, js[0])]
                    sm = pan["sm"][js[0]][0]
                    if len(js) == 2 and w0 == 512:
                        # pair occupies sm[0 : 512+w1] contiguously: one exp
                        regs = [(sm[:, 0:512 + w1],
                                 pan["pt"][:, off:off + 512 + w1], 512 + w1)]
                    else:
                        regs = [
                            (sm[:, pan["sm"][j][1]:pan["sm"][j][1] + pan["sm"][j][2]],
                             pan["pt"][:, strip_off[(p, j)]:
                                       strip_off[(p, j)] + pan["sm"][j][2]],
                             pan["sm"][j][2])
                            for j in js
                        ]
                    for smreg, ptreg, w in regs:
                        eng = pick(w * ACT_COL + ACT_FIX, w * DVE_COL + DVE_FIX)
                        if eng == "A":
                            nc.scalar.activation(ptreg, smreg, Exp, scale=SCALE)
                        else:
                            nc.vector.tensor_scalar(
                                ptreg.bitcast(i16), smreg,
                                A_EXP, B_EXP, Alu.mult, Alu.add,
                            )
                    for j in js:
                        pan["sm"].pop(j)
                        if j >= 4 * p:
                            off_j = strip_off[(p, j)]
                            pd = pan["pt"][:, off_j:off_j + 128]
                            nc.gpsimd.affine_select(
                                out=pd, in_=pd,
                                compare_op=Alu.is_ge,
                                fill=0.0, base=0,
                                # keep where (q - kv) >= 0
                                pattern=[[1, 128]], channel_multiplier=-1,
                            )

                def emit_pvs(u):
                    h, p, js = u
                    hd = heads[h]
                    pan = panels[(h, p)]
                    for j in js:
                        w = _width(p, j)
                        off = strip_off[(p, j)]
                        nc.tensor.matmul(
                            pan["oq"][:, 512 - w:512],
                            hd["vp"][:, j, :],
                            pan["pt"][:, off:off + w],
                            start=(j == 0),
                            stop=(j == 4 * p + 3),
                            skip_group_check=True,
                        )

                def emit_tail(h, p):
                    pan = panels.pop((h, p))
                    hd = heads[h]
                    od = odp.tile([D + 1, 512], f16)
                    if pick(570.0, 660.0) == "A":
                        nc.scalar.copy(od[:], pan["oq"][:])
                    else:
                        nc.vector.tensor_copy(od[:], pan["oq"][:])
                    # 66-col stride keeps each fp16 PSUM slice 4-byte aligned
                    ot = otp.tile([128, 4, D + 2], f16)
                    for t in range(4):
                        nc.tensor.transpose(
                            ot[:, t, :],
                            od[:, t * 128:(t + 1) * 128],
                            ident[0:D + 1, 0:D + 2],
                        )
                    r = rp.tile([128, 4, 1], f32)
                    nc.vector.reciprocal(r[:, :, 0], ot[:, :, D])
                    bal["D"] += 460.0
                    a, b = bass.broadcast_tensor_aps(ot[:, :, 0:D], r[:])
                    nc.vector.tensor_tensor(
                        hd["fin"][:, 4 * p:4 * p + 4, :], a, b, Alu.mult
                    )
                    if p == NPANEL - 1:
                        nc.sync.dma_start(o_d[h], hd["fin"][:])

                npre = min(io_bufs, HPC)
                for h in range(npre):
                    emit_loads(h)
                for n, u in enumerate(unit_list):
                    if n == 0:
                        emit_scores(u)
                    if n + 1 < len(unit_list):
                        u2 = unit_list[n + 1]
                        if u2[1] == 0 and u2[2][0] == 0 and u2[0] >= npre:
                            emit_loads(u2[0])
                        emit_scores(u2)
                    emit_exps(u)
                    while pending:
                        emit_tail(*pending.pop(0))
                    emit_pvs(u)
                    h, p, js = u
                    if js[-1] == 4 * p + 3:
                        if tail_delay and n + 1 < len(unit_list):
                            pending.append((h, p))
                        else:
                            emit_tail(h, p)
                while pending:
                    emit_tail(*pending.pop(0))

            if repeat == 1:
                emit_body()
            else:
                import concourse.mybir as _mb
                engs = (
                    _mb.EngineType.PE,
                    _mb.EngineType.Activation,
                    _mb.EngineType.DVE,
                    _mb.EngineType.SP,
                    _mb.EngineType.Pool,
                )
                with tc.For_i(0, repeat, 1, hint_engines=engs):
                    emit_body()

    nc.compile()
    return nc


def _get_program():
    if "nc" not in _cache:
        os.environ.setdefault("MYCRO_LOCAL_CACHE", "1")
        _cache["nc"] = build_program()
    return _cache["nc"]


def pack_inputs(q, k, v):
    """Host-side packing: fp16, q/k transposed to [d, t] and duplicated across
    both partition halves, V packed [128, kv-block, D+1] with a ones column."""
    q = np.asarray(q).reshape(B * H, T, D).astype(np.float16)
    k = np.asarray(k).reshape(B * H, T, D).astype(np.float16)
    qT = np.ascontiguousarray(q.transpose(0, 2, 1))
    kT = np.ascontiguousarray(k.transpose(0, 2, 1))
    qd = np.concatenate([qT, qT], axis=1)        # [BH, 128, T]
    kd = np.concatenate([kT, kT], axis=1)
    v = np.asarray(v).reshape(B * H, T // 128, 128, D).astype(np.float16)
    vp = np.ones((B * H, 128, T // 128, D + 1), np.float16)
    vp[:, :, :, 0:D] = v.transpose(0, 2, 1, 3)
    return qd, kd, vp


def unpack_output(raw):
    """[BH, 128, NQT, D] fp16 partition-major -> [B, H, T, D] fp32."""
    out = np.asarray(raw).transpose(0, 2, 1, 3)   # [BH, NQT, 128, D]
    return out.reshape(B, H, T, D).astype(np.float32)


def kernel(q, k, v):
    from concourse.bass_utils import run_bass_kernel_spmd

    qd, kd, vp = pack_inputs(q, k, v)
    nc = _get_program()
    in_maps = [
        {
            "qd": qd[c * HPC:(c + 1) * HPC],
            "kd": kd[c * HPC:(c + 1) * HPC],
            "v": vp[c * HPC:(c + 1) * HPC],
        }
        for c in range(NCORES)
    ]
    res = run_bass_kernel_spmd(nc, in_maps, list(range(NCORES)))
    kernel._last = res
    out = np.concatenate([res.results[c]["out"] for c in range(NCORES)], axis=0)
    return unpack_output(out)


# revision 14
# speedup vs baseline: 1.2314x; 1.2314x over previous
"""Causal multi-head attention (B=2, H=12, T=2048, D=64) on 8 Trainium2 NeuronCores.

Sharding: the 24 (batch, head) pairs are split 3-per-core across 8 cores.

Per head the device kernel works panel-major in transposed-score layout: the
16 q-tiles form 4 panels of 512 q columns. For panel p, kv strips j=0..4p+3
(width 512 for j<=4p, then 384/256/128) are processed as row-tiled PAIRS:

    S^T strips = K_j @ Q_panel   (PE, K=64 contraction). The two strips of a
        pair run CONCURRENTLY on array rows 0-63 / 64-127 (Q/K host-duplicated
        across both partition halves), writing the two 512-col banks of one
        [128,1024] PSUM mega tile.
    P^T = exp(S^T/8)             usually ONE instruction per pair, greedily
        load-balanced between ACT (true exp) and DVE (1-pass Schraudolph
        fast-exp: i16 = S*(128/ln2)+B, bitcast fp16, ~3% max err — the
        softmax ratio cancels most of it).
    diag blocks masked in-place on GPSIMD (affine_select, keep q >= kv)
    O'^T[65, 512] += [V_j | ones] @ P^T strip   (PE, N<=512 wide; row 64
        accumulates softmax denominators)
    tail per panel: O'^T -> SBUF fp16, 4 PE transposes (fp16, 66-col stride
        for PSUM alignment), DVE reciprocal of the denominator column,
        broadcast multiply into a per-head fp16 output buffer, one
        partition-major DMA out per head (host untangles the layout).

`repeat` > 1 wraps the body in a hardware For_i loop (timing aid only).

Self-contained: only imports numpy + the installed concourse/bass stack.
"""

import math
import os
import numpy as np

B, H, T, D = 2, 12, 2048, 64
NCORES = 8
HPC = (B * H) // NCORES      # heads per core = 3
NQT = T // 128               # 16 q tiles of 128 rows
NPANEL = 4                   # panels of 4 q-tiles (512 q cols)
SCALE = 1.0 / 8.0            # 1/sqrt(D)

# Schraudolph fast-exp constants (fp16 bit pattern via int16):
#   i16 = round(S_raw * (2^10/ln2 * SCALE) + B_EXP);  fp16<bits=i16> ~ exp(S_raw/8)
A_EXP = 1024.0 / math.log(2.0) * SCALE
B_EXP = 15315.5

# measured per-op engine costs (ns) for the ACT/DVE load balancer
ACT_COL = 0.845
DVE_COL = 0.98
ACT_FIX = 290.0
DVE_FIX = 250.0

_cache = {}


def _width(p, j):
    return 512 if j <= 4 * p else (4 * p + 4 - j) * 128


def build_program(io_bufs=2, mega_bufs=3, oq_bufs=1, ot_bufs=1, tail_delay=1,
                  exp_engine="both", no_mask=False, no_pack=False,
                  mask_mode="gpsimd", ablate=(), repeat=1):
    import concourse.bacc as bacc
    import concourse.bass as bass
    import concourse.mybir as mybir
    import concourse.tile as tile
    from concourse.masks import make_identity, make_upper_triangular

    f16 = mybir.dt.float16
    f32 = mybir.dt.float32
    i16 = mybir.dt.int16
    Exp = mybir.ActivationFunctionType.Exp
    Alu = mybir.AluOpType

    nc = bacc.Bacc(None)
    # q/k stored transposed [d, t], duplicated across both partition halves
    q_d = nc.dram_tensor("qd", [HPC, 128, T], f16, kind="ExternalInput")
    k_d = nc.dram_tensor("kd", [HPC, 128, T], f16, kind="ExternalInput")
    v_d = nc.dram_tensor("v", [HPC, 128, NQT, D + 1], f16, kind="ExternalInput")
    # partition-major: [head, q-row-in-tile, q-tile, d]; host untangles
    o_d = nc.dram_tensor("out", [HPC, 128, NQT, D], f16, kind="ExternalOutput")

    strip_off = {}
    pt_cols = {}
    for p in range(NPANEL):
        off = 0
        for j in range(4 * p + 4):
            strip_off[(p, j)] = off
            off += _width(p, j)
        pt_cols[p] = off

    # units: consecutive strip pairs
    unit_list = []
    for h in range(HPC):
        for p in range(NPANEL):
            js = list(range(4 * p + 4))
            for g in range(0, len(js), 2):
                unit_list.append((h, p, tuple(js[g:g + 2])))

    with tile.TileContext(nc) as tc:
        with (
            tc.tile_pool(name="consts", bufs=1) as consts,
            tc.tile_pool(name="qk", bufs=io_bufs) as qk,
            tc.tile_pool(name="vpool", bufs=io_bufs) as vpool,
            tc.tile_pool(name="ptpool", bufs=2) as ptpool,
            tc.tile_pool(name="odp", bufs=2) as odp,
            tc.tile_pool(name="rp", bufs=2) as rp,
            tc.tile_pool(name="finp", bufs=2) as finp,
            tc.tile_pool(name="smf", bufs=mega_bufs, space="PSUM") as smf,
            tc.tile_pool(name="oqp", bufs=oq_bufs, space="PSUM") as oqp,
            tc.tile_pool(name="otp", bufs=ot_bufs, space="PSUM") as otp,
        ):
            ident = consts.tile([128, 128], f16)
            make_identity(nc, ident[:])
            # strict-upper -MASKVAL: one PE accumulate-matmul masks a diag
            # block (exp maps masked scores to negligible subnormals)
            uneg = consts.tile([128, 128], f16)
            make_upper_triangular(nc, uneg[:], val=-600.0, diag=False)
            # Warm the ACT exp table set while the first DMAs are in flight.
            warm = consts.tile([128, 1], f32)
            nc.scalar.activation(warm[:], ident[:, 0:1], Exp)

            def emit_body():
                heads = {}
                panels = {}
                bal = {"A": 0.0, "D": 0.0}
                pending = []

                def pick(act_cost, dve_cost):
                    if exp_engine == "act":
                        return "A"
                    if exp_engine == "dve":
                        return "D"
                    if bal["A"] + act_cost <= bal["D"] + dve_cost:
                        bal["A"] += act_cost
                        return "A"
                    bal["D"] += dve_cost
                    return "D"

                def emit_loads(h, chunks=1):
                    qt = qk.tile([128, T], f16, tag="qt")
                    kt = qk.tile([128, T], f16, tag="kt")
                    cw = T // chunks
                    for c in range(0, T, cw):
                        nc.sync.dma_start(kt[:, c:c + cw], k_d[h, :, c:c + cw])
                        nc.sync.dma_start(qt[:, c:c + cw], q_d[h, :, c:c + cw])
                    vp = vpool.tile([128, NQT, D + 1], f16)
                    nc.sync.dma_start(vp[:], v_d[h])
                    fin = finp.tile([128, NQT, D], f16)
                    heads[h] = {"qt": qt, "kt": kt, "vp": vp, "fin": fin}

                def emit_scores(u):
                    h, p, js = u
                    qbase = 512 * p
                    hd = heads[h]
                    if (h, p) not in panels:
                        panels[(h, p)] = {
                            "pt": ptpool.tile([128, pt_cols[p]], f16,
                                              tag="pt", name="pt"),
                            "oq": oqp.tile([D + 1, 512], f32, tag="oq",
                                           name="oq"),
                            "sm": {},
                        }
                    pan = panels[(h, p)]
                    sm = smf.tile([128, 1024], f32, tag="sm", name="sm")
                    for side, j in enumerate(js):
                        w = _width(p, j)
                        rows = (slice(0, 64) if no_pack
                                else slice(64 * side, 64 * side + 64))
                        c0 = 512 * side
                        masked = j >= 4 * p and not no_mask and \
                            mask_mode == "matmul"
                        nc.tensor.matmul(
                            sm[:, c0:c0 + w],
                            hd["kt"][rows, j * 128:(j + 1) * 128],
                            hd["qt"][rows, qbase + 512 - w:qbase + 512],
                            start=True, stop=not masked,
                        )
                        if masked:
                            # diag block is the strip's first 128 cols
                            nc.tensor.matmul(
                                sm[:, c0:c0 + 128],
                                uneg[:],
                                ident[:],
                                start=False, stop=True,
                                skip_group_check=True,
                            )
                        pan["sm"][j] = (sm, c0, w)

                def emit_exps(u):
                    h, p, js = u
                    pan = panels[(h, p)]
                    w0 = _width(p, js[0])
                    w1 = _width(p, js[-1]) if len(js) > 1 else 0
                    off = strip_off[(p, js[0])]
                    sm = pan["sm"][js[0]][0]
                    if len(js) == 2 and w0 == 512:
                        # pair occupies sm[0 : 512+w1] contiguously: one exp
                        regs = [(sm[:, 0:512 + w1],
                                 pan["pt"][:, off:off + 512 + w1], 512 + w1)]
                    else:
                        regs = [
                            (sm[:, pan["sm"][j][1]:pan["sm"][j][1] + pan["sm"][j][2]],
                             pan["pt"][:, strip_off[(p, j)]:
                                       strip_off[(p, j)] + pan["sm"][j][2]],
                             pan["sm"][j][2])
                            for j in js
                        ]
                    # diag strips masked via the -600 matmul addend must use
                    # the true exp (the fast-exp bit trick would turn large
                    # negative scores into garbage negative fp16 values)
                    force_act = js[0] >= 4 * p and mask_mode == "matmul"
                    for smreg, ptreg, w in regs:
                        if "exp" in ablate:
                            break
                        if force_act:
                            eng = "A"
                            bal["A"] += w * ACT_COL + ACT_FIX
                        else:
                            eng = pick(w * ACT_COL + ACT_FIX,
                                       w * DVE_COL + DVE_FIX)
                        if eng == "A":
                            nc.scalar.activation(ptreg, smreg, Exp, scale=SCALE)
                        else:
                            nc.vector.tensor_scalar(
                                ptreg.bitcast(i16), smreg,
                                A_EXP, B_EXP, Alu.mult, Alu.add,
                            )
                    for j in js:
                        pan["sm"].pop(j)
                        if j >= 4 * p and not no_mask and mask_mode == "gpsimd":
                            off_j = strip_off[(p, j)]
                            pd = pan["pt"][:, off_j:off_j + 128]
                            nc.gpsimd.affine_select(
                                out=pd, in_=pd,
                                compare_op=Alu.is_ge,
                                fill=0.0, base=0,
                                # keep where (q - kv) >= 0
                                pattern=[[1, 128]], channel_multiplier=-1,
                            )

                def emit_pvs(u):
                    h, p, js = u
                    hd = heads[h]
                    pan = panels[(h, p)]
                    for j in js:
                        w = _width(p, j)
                        off = strip_off[(p, j)]
                        nc.tensor.matmul(
                            pan["oq"][:, 512 - w:512],
                            hd["vp"][:, j, :],
                            pan["pt"][:, off:off + w] if "pv1" not in ablate
                            else pan["pt"][:, 0:w],
                            start=(j == 0),
                            stop=(j == 4 * p + 3),
                            skip_group_check=True,
                        )

                def emit_tail(h, p):
                    pan = panels.pop((h, p))
                    hd = heads[h]
                    if "tail" in ablate:
                        if p == NPANEL - 1:
                            nc.sync.dma_start(o_d[h], hd["fin"][:])
                        return
                    od = odp.tile([D + 1, 512], f16)
                    if pick(675.0, 750.0) == "A":
                        nc.scalar.copy(od[:], pan["oq"][:])
                    else:
                        nc.vector.tensor_copy(od[:], pan["oq"][:])
                    # 66-col stride keeps each fp16 PSUM slice 4-byte aligned
                    ot = otp.tile([128, 4, D + 2], f16)
                    for t in range(4):
                        nc.tensor.transpose(
                            ot[:, t, :],
                            od[:, t * 128:(t + 1) * 128],
                            ident[0:D + 1, 0:D + 2],
                        )
                    r = rp.tile([128, 4, 1], f32)
                    nc.vector.reciprocal(r[:, :, 0], ot[:, :, D])
                    bal["D"] += 850.0
                    a, b = bass.broadcast_tensor_aps(ot[:, :, 0:D], r[:])
                    nc.vector.tensor_tensor(
                        hd["fin"][:, 4 * p:4 * p + 4, :], a, b, Alu.mult
                    )
                    if p == NPANEL - 1:
                        nc.sync.dma_start(o_d[h], hd["fin"][:])

                npre = min(io_bufs, HPC)
                for h in range(npre):
                    emit_loads(h)
                for n, u in enumerate(unit_list):
                    if n == 0:
                        emit_scores(u)
                    if n + 1 < len(unit_list):
                        u2 = unit_list[n + 1]
                        if u2[1] == 0 and u2[2][0] == 0 and u2[0] >= npre:
                            emit_loads(u2[0])
                        emit_scores(u2)
                    emit_exps(u)
                    while pending:
                        emit_tail(*pending.pop(0))
                    emit_pvs(u)
                    h, p, js = u
                    if js[-1] == 4 * p + 3:
                        if tail_delay and n + 1 < len(unit_list):
                            pending.append((h, p))
                        else:
                            emit_tail(h, p)
                while pending:
                    emit_tail(*pending.pop(0))

            if repeat == 1:
                emit_body()
            else:
                import concourse.mybir as _mb
                engs = (
                    _mb.EngineType.PE,
                    _mb.EngineType.Activation,
                    _mb.EngineType.DVE,
                    _mb.EngineType.SP,
                    _mb.EngineType.Pool,
                )
                with tc.For_i(0, repeat, 1, hint_engines=engs):
                    emit_body()

    nc.compile()
    return nc


def _get_program():
    if "nc" not in _cache:
        os.environ.setdefault("MYCRO_LOCAL_CACHE", "1")
        _cache["nc"] = build_program()
    return _cache["nc"]


def pack_inputs(q, k, v):
    """Host-side packing: fp16, q/k transposed to [d, t] and duplicated across
    both partition halves, V packed [128, kv-block, D+1] with a ones column."""
    q = np.asarray(q).reshape(B * H, T, D).astype(np.float16)
    k = np.asarray(k).reshape(B * H, T, D).astype(np.float16)
    qT = np.ascontiguousarray(q.transpose(0, 2, 1))
    kT = np.ascontiguousarray(k.transpose(0, 2, 1))
    qd = np.concatenate([qT, qT], axis=1)        # [BH, 128, T]
    kd = np.concatenate([kT, kT], axis=1)
    v = np.asarray(v).reshape(B * H, T // 128, 128, D).astype(np.float16)
    vp = np.ones((B * H, 128, T // 128, D + 1), np.float16)
    vp[:, :, :, 0:D] = v.transpose(0, 2, 1, 3)
    return qd, kd, vp


def unpack_output(raw):
    """[BH, 128, NQT, D] fp16 partition-major -> [B, H, T, D] fp32."""
    out = np.asarray(raw).transpose(0, 2, 1, 3)   # [BH, NQT, 128, D]
    return out.reshape(B, H, T, D).astype(np.float32)


def kernel(q, k, v):
    from concourse.bass_utils import run_bass_kernel_spmd

    qd, kd, vp = pack_inputs(q, k, v)
    nc = _get_program()
    in_maps = [
        {
            "qd": qd[c * HPC:(c + 1) * HPC],
            "kd": kd[c * HPC:(c + 1) * HPC],
            "v": vp[c * HPC:(c + 1) * HPC],
        }
        for c in range(NCORES)
    ]
    res = run_bass_kernel_spmd(nc, in_maps, list(range(NCORES)))
    kernel._last = res
    out = np.concatenate([res.results[c]["out"] for c in range(NCORES)], axis=0)
    return unpack_output(out)


# revision 15
# speedup vs baseline: 1.3789x; 1.1197x over previous
"""Causal multi-head attention (B=2, H=12, T=2048, D=64) on 8 Trainium2 NeuronCores.

Sharding: the 24 (batch, head) pairs are split 3-per-core across 8 cores.

Per head the device kernel works panel-major in transposed-score layout: the
16 q-tiles form 4 panels of 512 q columns. For panel p, kv strips j=0..4p+3
(width 512 for j<=4p, then 384/256/128) are processed as row-tiled PAIRS:

    S^T strips = K_j @ Q_panel   (PE, K=64 contraction). The two strips of a
        pair run CONCURRENTLY on array rows 0-63 / 64-127 (Q/K host-duplicated
        across both partition halves), writing the two 512-col banks of one
        [128,1024] PSUM mega tile.
    P^T = exp(S^T/8)             usually ONE instruction per pair, greedily
        load-balanced between ACT (true exp) and DVE (1-pass Schraudolph
        fast-exp: i16 = S*(128/ln2)+B, bitcast fp16, ~3% max err — the
        softmax ratio cancels most of it).
    diag blocks masked in-place on GPSIMD (affine_select, keep q >= kv)
    O'^T[65, 512] += [V_j | ones] @ P^T strip   (PE, N<=512 wide; row 64
        accumulates softmax denominators)
    tail per panel: O'^T -> SBUF fp16, 4 PE transposes (fp16, 66-col stride
        for PSUM alignment), DVE reciprocal of the denominator column,
        broadcast multiply into a per-head fp16 output buffer, one
        partition-major DMA out per head (host untangles the layout).

`repeat` > 1 wraps the body in a hardware For_i loop (timing aid only).

Self-contained: only imports numpy + the installed concourse/bass stack.
"""

import math
import os
import numpy as np

B, H, T, D = 2, 12, 2048, 64
NCORES = 8
HPC = (B * H) // NCORES      # heads per core = 3
NQT = T // 128               # 16 q tiles of 128 rows
NPANEL = 4                   # panels of 4 q-tiles (512 q cols)
SCALE = 1.0 / 8.0            # 1/sqrt(D)

# Schraudolph fast-exp constants (fp16 bit pattern via int16):
#   i16 = round(S_raw * (2^10/ln2 * SCALE) + B_EXP);  fp16<bits=i16> ~ exp(S_raw/8)
A_EXP = 1024.0 / math.log(2.0) * SCALE
B_EXP = 15315.5

# measured per-op engine costs (ns) for the ACT/DVE load balancer
ACT_COL = 0.845
DVE_COL = 0.98
ACT_FIX = 290.0
DVE_FIX = 250.0

_cache = {}


def _width(p, j):
    return 512 if j <= 4 * p else (4 * p + 4 - j) * 128


def build_program(io_bufs=2, mega_bufs=3, oq_bufs=1, ot_bufs=1, tail_delay=1,
                  exp_engine="both", no_mask=False, no_pack=False,
                  mask_mode="gpsimd", ablate=(), repeat=1):
    import concourse.bacc as bacc
    import concourse.bass as bass
    import concourse.mybir as mybir
    import concourse.tile as tile
    from concourse.masks import make_identity, make_upper_triangular

    f16 = mybir.dt.float16
    f32 = mybir.dt.float32
    i16 = mybir.dt.int16
    Exp = mybir.ActivationFunctionType.Exp
    Alu = mybir.AluOpType

    nc = bacc.Bacc(None)
    # q/k stored transposed [d, t], duplicated across both partition halves
    q_d = nc.dram_tensor("qd", [HPC, 128, T], f16, kind="ExternalInput")
    k_d = nc.dram_tensor("kd", [HPC, 128, T], f16, kind="ExternalInput")
    v_d = nc.dram_tensor("v", [HPC, 128, NQT, D + 1], f16, kind="ExternalInput")
    # partition-major: [head, q-row-in-tile, q-tile, d]; host untangles
    o_d = nc.dram_tensor("out", [HPC, 128, NQT, D], f16, kind="ExternalOutput")

    strip_off = {}
    pt_cols = {}
    for p in range(NPANEL):
        off = 0
        for j in range(4 * p + 4):
            strip_off[(p, j)] = off
            off += _width(p, j)
        pt_cols[p] = off

    # units: consecutive strip pairs
    unit_list = []
    for h in range(HPC):
        for p in range(NPANEL):
            js = list(range(4 * p + 4))
            for g in range(0, len(js), 2):
                unit_list.append((h, p, tuple(js[g:g + 2])))

    with tile.TileContext(nc) as tc:
        with (
            tc.tile_pool(name="consts", bufs=1) as consts,
            tc.tile_pool(name="qk", bufs=io_bufs) as qk,
            tc.tile_pool(name="vpool", bufs=io_bufs) as vpool,
            tc.tile_pool(name="ptpool", bufs=2) as ptpool,
            tc.tile_pool(name="odp", bufs=2) as odp,
            tc.tile_pool(name="rp", bufs=2) as rp,
            tc.tile_pool(name="finp", bufs=2) as finp,
            tc.tile_pool(name="smf", bufs=mega_bufs, space="PSUM") as smf,
            tc.tile_pool(name="oqp", bufs=oq_bufs, space="PSUM") as oqp,
            tc.tile_pool(name="otp", bufs=ot_bufs, space="PSUM") as otp,
        ):
            ident = consts.tile([128, 128], f16)
            make_identity(nc, ident[:])
            # strict-upper -MASKVAL: one PE accumulate-matmul masks a diag
            # block (exp maps masked scores to negligible subnormals)
            uneg = consts.tile([128, 128], f16)
            make_upper_triangular(nc, uneg[:], val=-600.0, diag=False)
            # Warm the ACT exp table set while the first DMAs are in flight.
            warm = consts.tile([128, 1], f32)
            nc.scalar.activation(warm[:], ident[:, 0:1], Exp)

            def emit_body():
                heads = {}
                panels = {}
                bal = {"A": 0.0, "D": 0.0}
                pending = []

                def pick(act_cost, dve_cost):
                    if exp_engine == "act":
                        return "A"
                    if exp_engine == "dve":
                        return "D"
                    if bal["A"] + act_cost <= bal["D"] + dve_cost:
                        bal["A"] += act_cost
                        return "A"
                    bal["D"] += dve_cost
                    return "D"

                def emit_loads(h, chunks=1):
                    qt = qk.tile([128, T], f16, tag="qt")
                    kt = qk.tile([128, T], f16, tag="kt")
                    cw = T // chunks
                    for c in range(0, T, cw):
                        nc.sync.dma_start(kt[:, c:c + cw], k_d[h, :, c:c + cw])
                        nc.sync.dma_start(qt[:, c:c + cw], q_d[h, :, c:c + cw])
                    vp = vpool.tile([128, NQT, D + 1], f16)
                    nc.sync.dma_start(vp[:], v_d[h])
                    fin = finp.tile([128, NQT, D], f16)
                    heads[h] = {"qt": qt, "kt": kt, "vp": vp, "fin": fin}

                def emit_scores(u):
                    h, p, js = u
                    qbase = 512 * p
                    hd = heads[h]
                    if (h, p) not in panels:
                        panels[(h, p)] = {
                            "pt": ptpool.tile([128, pt_cols[p]], f16,
                                              tag="pt", name="pt"),
                            "oq": oqp.tile([D + 1, 512], f32, tag="oq",
                                           name="oq"),
                            "sm": {},
                        }
                    pan = panels[(h, p)]
                    sm = smf.tile([128, 1024], f32, tag="sm", name="sm")
                    for side, j in enumerate(js):
                        w = _width(p, j)
                        rows = (slice(0, 64) if no_pack
                                else slice(64 * side, 64 * side + 64))
                        c0 = 512 * side
                        masked = j >= 4 * p and not no_mask and \
                            mask_mode == "matmul"
                        nc.tensor.matmul(
                            sm[:, c0:c0 + w],
                            hd["kt"][rows, j * 128:(j + 1) * 128],
                            hd["qt"][rows, qbase + 512 - w:qbase + 512],
                            start=True, stop=not masked,
                        )
                        if masked:
                            # diag block is the strip's first 128 cols
                            nc.tensor.matmul(
                                sm[:, c0:c0 + 128],
                                uneg[:],
                                ident[:],
                                start=False, stop=True,
                                skip_group_check=True,
                            )
                        pan["sm"][j] = (sm, c0, w)

                def emit_exps(u):
                    h, p, js = u
                    pan = panels[(h, p)]
                    w0 = _width(p, js[0])
                    w1 = _width(p, js[-1]) if len(js) > 1 else 0
                    off = strip_off[(p, js[0])]
                    sm = pan["sm"][js[0]][0]
                    if len(js) == 2 and w0 == 512:
                        # pair occupies sm[0 : 512+w1] contiguously: one exp
                        regs = [(sm[:, 0:512 + w1],
                                 pan["pt"][:, off:off + 512 + w1], 512 + w1)]
                    else:
                        regs = [
                            (sm[:, pan["sm"][j][1]:pan["sm"][j][1] + pan["sm"][j][2]],
                             pan["pt"][:, strip_off[(p, j)]:
                                       strip_off[(p, j)] + pan["sm"][j][2]],
                             pan["sm"][j][2])
                            for j in js
                        ]
                    # diag strips masked via the -600 matmul addend must use
                    # the true exp (the fast-exp bit trick would turn large
                    # negative scores into garbage negative fp16 values)
                    force_act = js[0] >= 4 * p and mask_mode == "matmul"
                    for smreg, ptreg, w in regs:
                        if "exp" in ablate:
                            break
                        if force_act:
                            eng = "A"
                            bal["A"] += w * ACT_COL + ACT_FIX
                        else:
                            eng = pick(w * ACT_COL + ACT_FIX,
                                       w * DVE_COL + DVE_FIX)
                        if eng == "A":
                            nc.scalar.activation(ptreg, smreg, Exp, scale=SCALE)
                        else:
                            nc.vector.tensor_scalar(
                                ptreg.bitcast(i16), smreg,
                                A_EXP, B_EXP, Alu.mult, Alu.add,
                            )
                    for j in js:
                        pan["sm"].pop(j)
                        if j >= 4 * p and not no_mask and mask_mode == "gpsimd":
                            off_j = strip_off[(p, j)]
                            pd = pan["pt"][:, off_j:off_j + 128]
                            nc.gpsimd.affine_select(
                                out=pd, in_=pd,
                                compare_op=Alu.is_ge,
                                fill=0.0, base=0,
                                # keep where (q - kv) >= 0
                                pattern=[[1, 128]], channel_multiplier=-1,
                            )

                def emit_pvs(u):
                    h, p, js = u
                    hd = heads[h]
                    pan = panels[(h, p)]
                    for j in js:
                        w = _width(p, j)
                        off = strip_off[(p, j)]
                        nc.tensor.matmul(
                            pan["oq"][:, 512 - w:512],
                            hd["vp"][:, j, :],
                            pan["pt"][:, off:off + w] if "pv1" not in ablate
                            else pan["pt"][:, 0:w],
                            start=(j == 0),
                            stop=(j == 4 * p + 3),
                            skip_group_check=True,
                        )

                def emit_tail1(h, p):
                    # drain O' PSUM -> SBUF; frees the oq ring slot
                    pan = panels.pop((h, p))
                    hd = heads[h]
                    if "tail" in ablate:
                        if p == NPANEL - 1:
                            nc.sync.dma_start(o_d[h], hd["fin"][:])
                        return None
                    od = odp.tile([D + 1, 512], f16)
                    if pick(675.0, 750.0) == "A":
                        nc.scalar.copy(od[:], pan["oq"][:])
                    else:
                        nc.vector.tensor_copy(od[:], pan["oq"][:])
                    return (h, p, od)

                def emit_tail2(h, p, od):
                    # transpose + normalize, emitted a couple of units after
                    # the od copy so the PE never waits on it
                    hd = heads[h]
                    # 66-col stride keeps each fp16 PSUM slice 4-byte aligned
                    ot = otp.tile([128, 4, D + 2], f16)
                    for t in range(4):
                        nc.tensor.transpose(
                            ot[:, t, :],
                            od[:, t * 128:(t + 1) * 128],
                            ident[0:D + 1, 0:D + 2],
                        )
                    r = rp.tile([128, 4, 1], f32)
                    nc.vector.reciprocal(r[:, :, 0], ot[:, :, D])
                    bal["D"] += 850.0
                    a, b = bass.broadcast_tensor_aps(ot[:, :, 0:D], r[:])
                    nc.vector.tensor_tensor(
                        hd["fin"][:, 4 * p:4 * p + 4, :], a, b, Alu.mult
                    )
                    if p == NPANEL - 1:
                        nc.sync.dma_start(o_d[h], hd["fin"][:])

                npre = min(io_bufs, HPC)
                for h in range(npre):
                    emit_loads(h)
                nu = len(unit_list)
                pending2 = []

                def maybe_scores(m):
                    if m < nu:
                        um = unit_list[m]
                        if um[1] == 0 and um[2][0] == 0 and um[0] >= npre:
                            emit_loads(um[0])
                        emit_scores(um)

                maybe_scores(0)
                maybe_scores(1)
                if nu:
                    emit_exps(unit_list[0])
                for n, u in enumerate(unit_list):
                    maybe_scores(n + 2)
                    if n + 1 < nu:
                        emit_exps(unit_list[n + 1])
                    while len(pending2) > (2 if n + 1 < nu else 0):
                        emit_tail2(*pending2.pop(0))
                    emit_pvs(u)
                    h, p, js = u
                    if js[-1] == 4 * p + 3:
                        t1 = emit_tail1(h, p)
                        if t1 is not None:
                            pending2.append(t1)
                while pending2:
                    emit_tail2(*pending2.pop(0))

            if repeat == 1:
                emit_body()
            else:
                import concourse.mybir as _mb
                engs = (
                    _mb.EngineType.PE,
                    _mb.EngineType.Activation,
                    _mb.EngineType.DVE,
                    _mb.EngineType.SP,
                    _mb.EngineType.Pool,
                )
                with tc.For_i(0, repeat, 1, hint_engines=engs):
                    emit_body()

    nc.compile()
    return nc


def _get_program():
    if "nc" not in _cache:
        os.environ.setdefault("MYCRO_LOCAL_CACHE", "1")
        _cache["nc"] = build_program()
    return _cache["nc"]


def pack_inputs(q, k, v):
    """Host-side packing: fp16, q/k transposed to [d, t] and duplicated across
    both partition halves, V packed [128, kv-block, D+1] with a ones column."""
    q = np.asarray(q).reshape(B * H, T, D).astype(np.float16)
    k = np.asarray(k).reshape(B * H, T, D).astype(np.float16)
    qT = np.ascontiguousarray(q.transpose(0, 2, 1))
    kT = np.ascontiguousarray(k.transpose(0, 2, 1))
    qd = np.concatenate([qT, qT], axis=1)        # [BH, 128, T]
    kd = np.concatenate([kT, kT], axis=1)
    v = np.asarray(v).reshape(B * H, T // 128, 128, D).astype(np.float16)
    vp = np.ones((B * H, 128, T // 128, D + 1), np.float16)
    vp[:, :, :, 0:D] = v.transpose(0, 2, 1, 3)
    return qd, kd, vp


def unpack_output(raw):
    """[BH, 128, NQT, D] fp16 partition-major -> [B, H, T, D] fp32."""
    out = np.asarray(raw).transpose(0, 2, 1, 3)   # [BH, NQT, 128, D]
    return out.reshape(B, H, T, D).astype(np.float32)


def kernel(q, k, v):
    from concourse.bass_utils import run_bass_kernel_spmd

    qd, kd, vp = pack_inputs(q, k, v)
    nc = _get_program()
    in_maps = [
        {
            "qd": qd[c * HPC:(c + 1) * HPC],
            "kd": kd[c * HPC:(c + 1) * HPC],
            "v": vp[c * HPC:(c + 1) * HPC],
        }
        for c in range(NCORES)
    ]
    res = run_bass_kernel_spmd(nc, in_maps, list(range(NCORES)))
    kernel._last = res
    out = np.concatenate([res.results[c]["out"] for c in range(NCORES)], axis=0)
    return unpack_output(out)
